# revision 2
# baseline (speedup 1.0000x reference)
"""Conv4d (Strang rearrange) Trainium2 kernel — raw bacc pipeline, v2.

Math: Strang-rearranged 4D conv == 3x3 conv over (D1,D2) with 16 input
channels (cin x h-parity x w-parity) per shift, batched over pixel dims.
Per core (8 = B x D1-half): 32 groups g=(u, rnd-half of V), each 9 (ku,kv)
shift-packs of 4 column-tiled matmuls (K=128 block-diag weights, M=32/strip).

v2 changes vs baseline:
  - zero padding row dropped: upper-half cores are D1-flipped on host (weights
    ku-flipped to match) so every core stores 17 real rows; (u==0, ku==0)
    matmuls are skipped.
  - whole-row input DMAs (rows 2..16), rows 0..1 still split a/b for an early
    first matmul.
  - output ys is partition-major [128, 32 groups, 1KiB]; output DMAs batched
    4 groups (4KiB/partition contiguous) from a double-buffered SBUF tile.
  - prologue ordering: the rows needed by group 0/1 and the weights issue
    first, split across the two HWDGE rings (sync + scalar).
"""

from contextlib import ExitStack

import ml_dtypes
import numpy as np

import concourse.bass as bass
from concourse import bacc, mybir
from concourse.bass_utils import run_bass_kernel_spmd

F16 = mybir.dt.float16
BF16 = mybir.dt.bfloat16
F32 = mybir.dt.float32

B, CIN, COUT = 4, 4, 4
D1, D2, H, W = 32, 32, 64, 64
U = 16
R = U + 1  # 17 real rows (zero pad row handled by skipping matmuls)
V = D2
I, J = H // 2, W // 2
IB, IO = 8, 4
VBS = 4
NCORES = 8
NZ, NPS = 6, 6
OB = 4   # output groups batched per DMA
NOUT = 2  # double-buffered output tiles
NG = 2 * U  # 32 groups

SHIFTS = [(ku, kv) for kv in (1, 0, 2) for ku in range(3)]
NSHIFT = len(SHIFTS)

SYNC_ROWS = (3, 5, 7, 9, 11, 13)
SCALAR_ROWS = (2, 4, 6, 8, 10, 12, 14, 15, 16)


def _host_weights(w, b):
    wbd = np.zeros((NSHIFT, 128, 32), np.float32)
    w = np.asarray(w, np.float32)
    for s, (ku, kv) in enumerate(SHIFTS):
        for kh in range(2):
            for kw in range(2):
                for ib in range(IB):
                    wbd[s, kh * 16 + kw * 8 + ib : 128 : 32, ib : 32 : 8] = (
                        w[:, :, ku, kv, kh, kw].T
                    )
    wbd_t = np.ascontiguousarray(wbd.transpose(1, 0, 2)).astype(ml_dtypes.bfloat16)
    bias = np.tile(np.repeat(np.asarray(b, np.float32), IB), 4).reshape(128, 1)
    return wbd_t, bias


def _host_shard(x):
    xp = np.pad(np.asarray(x, np.float32), ((0, 0), (0, 0), (1, 1), (0, 0), (0, 0), (0, 0)))
    shards = []
    for core in range(NCORES):
        bb, half = divmod(core, 2)
        if half == 0:
            xs = xp[bb, :, 1:18]                      # padded rows 1..17
        else:
            xs = xp[bb, :, 16:33][:, ::-1]            # padded rows 32..16 (D1-flipped)
        xs = xs.reshape(CIN, R, V, IO, IB, 2, J, 2)
        xs = xs.transpose(1, 0, 5, 7, 4, 2, 3, 6).astype(ml_dtypes.bfloat16)
        shards.append(np.ascontiguousarray(xs).reshape(R, 128, V, IO, J))
    return shards


def _shifts_for(u):
    return [s for s, (ku, kv) in enumerate(SHIFTS) if not (u == 0 and ku == 0)]


def _build_program():
    nc = bacc.Bacc("TRN2", target_bir_lowering=False, debug=False)
    xs = nc.dram_tensor("xs", [R, 128, V, IO, J], BF16, kind="ExternalInput").ap()
    wbd = nc.dram_tensor("wbd", [128, NSHIFT, 32], BF16, kind="ExternalInput").ap()
    bias = nc.dram_tensor("bias", [128, 1], F32, kind="ExternalInput").ap()
    ys = nc.dram_tensor("ys", [128, NG, VBS, IO, J], F16, kind="ExternalOutput").ap()

    with ExitStack() as ctx:
        zt = [ctx.enter_context(nc.sbuf_tensor(f"z{i}", [128, V, IO, J], BF16)) for i in range(NZ)]
        wt = ctx.enter_context(nc.sbuf_tensor("wt", [128, NSHIFT, 32], BF16))
        bt = ctx.enter_context(nc.sbuf_tensor("bt", [128, 1], F32))
        ot = [ctx.enter_context(nc.sbuf_tensor(f"ot{i}", [128, OB, VBS, IO, J], F16)) for i in range(NOUT)]
        ps = [ctx.enter_context(nc.psum_tensor(f"ps{i}", [128, VBS, IO, J], F32)) for i in range(NPS)]
        sem_a = [ctx.enter_context(nc.semaphore(f"sem_a{r}")) for r in range(2)]
        sem_b = [ctx.enter_context(nc.semaphore(f"sem_b{r}")) for r in range(2)]
        sem_z = [ctx.enter_context(nc.semaphore(f"sem_z{r}")) for r in range(2, R)]
        sem_w = ctx.enter_context(nc.semaphore("sem_w"))
        sem_bias = ctx.enter_context(nc.semaphore("sem_bias"))
        sem_mm = ctx.enter_context(nc.semaphore("sem_mm"))
        sem_act = ctx.enter_context(nc.semaphore("sem_act"))
        sem_ob = [ctx.enter_context(nc.semaphore(f"sem_ob{i}")) for i in range(NOUT)]
        blk_ctx = nc.Block()
        block = blk_ctx.__enter__()

        # rows >= NZ reuse a ring slot last read by group 2*(r-6)+3; wait mm >= 2r-8
        def row_wait(eng, r):
            if r >= NZ:
                eng.wait_ge(sem_mm, 2 * r - 8)

        @block.sync
        def _(sync):
            sync.dma_start(zt[0][:, 0:17], xs[0, :, 0:17]).then_inc(sem_a[0], 16)
            sync.dma_start(zt[1][:, 0:17], xs[1, :, 0:17]).then_inc(sem_a[1], 16)
            sync.dma_start(wt[:], wbd[:]).then_inc(sem_w, 16)
            rows = list(SYNC_ROWS)
            for bo in range(NG // OB):
                # rows whose WAR trigger (mm >= 2r-8) precedes this batch's
                # act trigger (act >= 4*bo+4, implying mm >= 4*bo+4)
                while rows and (rows[0] < NZ or 2 * rows[0] - 8 <= 4 * bo + 4):
                    r = rows.pop(0)
                    row_wait(sync, r)
                    sync.dma_start(zt[r % NZ][:], xs[r]).then_inc(sem_z[r - 2], 16)
                sync.wait_ge(sem_act, OB * bo + OB)
                sync.dma_start(ys[:, OB * bo : OB * bo + OB], ot[bo % NOUT][:]).then_inc(
                    sem_ob[bo % NOUT], 16
                )
            finals = [(sem_w, 16), (sem_bias, 16), (sem_mm, NG), (sem_act, NG)]
            finals += [(s, 16 * (NG // OB // NOUT)) for s in sem_ob]
            for s, v in finals:
                sync.wait_ge(s, v)

        @block.tensor
        def _(tensor):
            for g in range(NG):
                u, rnd = divmod(g, 2)
                if g == 0:
                    tensor.wait_ge(sem_w, 16)
                    tensor.wait_ge(sem_a[0], 16)
                    tensor.wait_ge(sem_a[1], 16)
                if g == 1:
                    tensor.wait_ge(sem_b[0], 16)
                    tensor.wait_ge(sem_b[1], 16)
                if rnd == 0 and u >= 1:
                    tensor.wait_ge(sem_z[u - 1], 16)  # whole row u+1
                if g >= NPS:
                    tensor.wait_ge(sem_act, g - NPS + 1)
                psg = ps[g % NPS]
                valid = _shifts_for(u)
                last = None
                for s in valid:
                    ku, kv = SHIFTS[s]
                    for c in range(4):
                        v0 = (rnd * 4 + c) * VBS
                        vv0 = max(0, 1 - kv - v0)
                        vv1 = min(VBS, V + 1 - kv - v0)
                        a = v0 + vv0 + kv - 1
                        last = nc.tensor.matmul(
                            psg[c * 32 : (c + 1) * 32, vv0:vv1, :, :],
                            wt[:, s, :],
                            zt[(u + ku - 1) % NZ][:, a : a + (vv1 - vv0), :, :],
                            start=(s == valid[0]),
                            stop=(s == valid[-1]),
                            skip_group_check=True,
                            tile_position=(0, c * 32),
                        )
                last.then_inc(sem_mm)

        @block.scalar
        def _(scalar):
            scalar.dma_start(zt[0][:, 17:V], xs[0, :, 17:V]).then_inc(sem_b[0], 16)
            scalar.dma_start(bt[:], bias[:]).then_inc(sem_bias, 16)
            scalar.dma_start(zt[1][:, 17:V], xs[1, :, 17:V]).then_inc(sem_b[1], 16)
            rows = list(SCALAR_ROWS)
            for g in range(NG):
                while rows and (rows[0] < NZ or 2 * rows[0] - 8 <= g + 1):
                    r = rows.pop(0)
                    row_wait(scalar, r)
                    scalar.dma_start(zt[r % NZ][:], xs[r]).then_inc(sem_z[r - 2], 16)
                scalar.wait_ge(sem_mm, g + 1)
                if g == 0:
                    scalar.wait_ge(sem_bias, 16)
                bo = g // OB
                if bo >= NOUT and g % OB == 0:
                    scalar.wait_ge(sem_ob[bo % NOUT], 16 * (bo // NOUT))
                nc.scalar.activation(
                    ot[bo % NOUT][:, g % OB],
                    ps[g % NPS][:],
                    mybir.ActivationFunctionType.Identity,
                    bias=bt[:],
                ).then_inc(sem_act)

        blk_ctx.__exit__(None, None, None)

    nc.compile()
    return nc


def _unshard(results):
    y = np.empty((B, COUT, D1, D2, I, J), np.float32)
    for core in range(NCORES):
        bb, half = divmod(core, 2)
        arr = results[core]["ys"].astype(np.float32)
        # [128, g=(u,rnd), vb, io, j] ; p = 32c + 8cout + ib
        arr = arr.reshape(4, COUT, IB, U, 2, VBS, IO, J)
        arr = arr.transpose(1, 3, 4, 0, 5, 6, 2, 7)  # cout,u,rnd,c,vb,io,ib,j
        arr = arr.reshape(COUT, U, V, I, J)
        if half == 1:
            arr = arr[:, ::-1]
        y[bb, :, half * U : (half + 1) * U] = arr
    return y


TRACE = False
LAST_RESULT = [None]


def kernel(x, w, b, _cache={}):
    if "nc" not in _cache:
        _cache["nc"] = _build_program()
    nc = _cache["nc"]
    wbd_t, bias = _host_weights(w, b)
    wbd_f, bias_f = _host_weights(np.asarray(w)[:, :, ::-1], b)
    shards = _host_shard(x)
    in_maps = []
    for core in range(NCORES):
        half = core % 2
        in_maps.append({
            "xs": shards[core],
            "wbd": wbd_f if half == 1 else wbd_t,
            "bias": bias_f if half == 1 else bias,
        })
    res = run_bass_kernel_spmd(nc, in_maps, list(range(NCORES)), trace=TRACE)
    LAST_RESULT[0] = res
    return _unshard(res.results)


# revision 3
# speedup vs baseline: 1.0931x; 1.0931x over previous
"""Conv4d (Strang rearrange) Trainium2 kernel — raw bacc pipeline, v3.

Math: Strang-rearranged 4D conv == 3x3 conv over (D1,D2) with 16 input
channels (cin x h-parity x w-parity) per shift, batched over pixel dims.
Per core (8 = B x D1-half): 32 groups g=(u, rnd-half of V), each 9 (ku,kv)
shift-packs of 4 column-tiled matmuls (K=128 block-diag weights, M=32/strip).

v3 pipeline:
  - zero padding row dropped: upper-half cores are D1-flipped on host
    (weights ku-flipped) so every core stores 17 real rows; (u==0, ku==0)
    matmuls are skipped.
  - rows split a/b (v 0:17 / 17:32); even rows on the sync HWDGE ring, odd
    rows on the scalar ring, so consecutive rows stream concurrently and the
    sync ring carries reads only.
  - 8 z ring slots + 8 psum banks for prefetch slack (rows 0..7 ungated).
  - output ys is partition-major [128, 32, 1KiB]; outputs batched 4 groups
    (4KiB/partition contiguous) from double-buffered SBUF tiles, issued on
    the scalar ring right after the batch's last activation.
"""

from contextlib import ExitStack

import ml_dtypes
import numpy as np

import concourse.bass as bass
from concourse import bacc, mybir
from concourse.bass_utils import run_bass_kernel_spmd

F16 = mybir.dt.float16
BF16 = mybir.dt.bfloat16
F32 = mybir.dt.float32

B, CIN, COUT = 4, 4, 4
D1, D2, H, W = 32, 32, 64, 64
U = 16
R = U + 1  # 17 real rows
V = D2
I, J = H // 2, W // 2
IB, IO = 8, 4
VBS = 4
NCORES = 8
NZ, NPS = 8, 8
OB = 4    # output groups batched per DMA
NOUT = 2  # double-buffered output tiles
NG = 2 * U  # 32 groups
VA = 17   # a-half v range [0, VA), b-half [VA, V)

SHIFTS = [(ku, kv) for kv in (1, 0, 2) for ku in range(3)]
NSHIFT = len(SHIFTS)


def _host_weights(w, b):
    wbd = np.zeros((NSHIFT, 128, 32), np.float32)
    w = np.asarray(w, np.float32)
    for s, (ku, kv) in enumerate(SHIFTS):
        for kh in range(2):
            for kw in range(2):
                for ib in range(IB):
                    wbd[s, kh * 16 + kw * 8 + ib : 128 : 32, ib : 32 : 8] = (
                        w[:, :, ku, kv, kh, kw].T
                    )
    wbd_t = np.ascontiguousarray(wbd.transpose(1, 0, 2)).astype(ml_dtypes.bfloat16)
    bias = np.tile(np.repeat(np.asarray(b, np.float32), IB), 4).reshape(128, 1)
    return wbd_t, bias


def _host_shard(x):
    xp = np.pad(np.asarray(x, np.float32), ((0, 0), (0, 0), (1, 1), (0, 0), (0, 0), (0, 0)))
    shards = []
    for core in range(NCORES):
        bb, half = divmod(core, 2)
        if half == 0:
            xs = xp[bb, :, 1:18]            # padded rows 1..17
        else:
            xs = xp[bb, :, 16:33][:, ::-1]  # padded rows 32..16 (D1-flipped)
        xs = xs.reshape(CIN, R, V, IO, IB, 2, J, 2)
        xs = xs.transpose(1, 0, 5, 7, 4, 2, 3, 6).astype(ml_dtypes.bfloat16)
        shards.append(np.ascontiguousarray(xs).reshape(R, 128, V, IO, J))
    return shards


def _shifts_for(u):
    return [s for s, (ku, kv) in enumerate(SHIFTS) if not (u == 0 and ku == 0)]


def _row_gate(r):
    """mm count required before row r may overwrite its ring slot."""
    return max(0, 2 * r - 2 * NZ + 4)


def _build_program():
    nc = bacc.Bacc("TRN2", target_bir_lowering=False, debug=False)
    xs = nc.dram_tensor("xs", [R, 128, V, IO, J], BF16, kind="ExternalInput").ap()
    wbd = nc.dram_tensor("wbd", [128, NSHIFT, 32], BF16, kind="ExternalInput").ap()
    bias = nc.dram_tensor("bias", [128, 1], F32, kind="ExternalInput").ap()
    ys = nc.dram_tensor("ys", [128, NG, VBS, IO, J], F16, kind="ExternalOutput").ap()

    with ExitStack() as ctx:
        zt = [ctx.enter_context(nc.sbuf_tensor(f"z{i}", [128, V, IO, J], BF16)) for i in range(NZ)]
        wt = ctx.enter_context(nc.sbuf_tensor("wt", [128, NSHIFT, 32], BF16))
        bt = ctx.enter_context(nc.sbuf_tensor("bt", [128, 1], F32))
        ot = [ctx.enter_context(nc.sbuf_tensor(f"ot{i}", [128, OB, VBS, IO, J], F16)) for i in range(NOUT)]
        ps = [ctx.enter_context(nc.psum_tensor(f"ps{i}", [128, VBS, IO, J], F32)) for i in range(NPS)]
        sem_za = [ctx.enter_context(nc.semaphore(f"sem_za{r}")) for r in range(R)]
        sem_zb = [ctx.enter_context(nc.semaphore(f"sem_zb{r}")) for r in range(R)]
        sem_w = ctx.enter_context(nc.semaphore("sem_w"))
        sem_bias = ctx.enter_context(nc.semaphore("sem_bias"))
        sem_mm = ctx.enter_context(nc.semaphore("sem_mm"))
        sem_act = ctx.enter_context(nc.semaphore("sem_act"))
        sem_ob = [ctx.enter_context(nc.semaphore(f"sem_ob{i}")) for i in range(NOUT)]
        blk_ctx = nc.Block()
        block = blk_ctx.__enter__()

        def issue_row_half(eng, r, half, gated):
            if gated and _row_gate(r) > 0:
                eng.wait_ge(sem_mm, _row_gate(r))
            if half == 0:
                eng.dma_start(zt[r % NZ][:, 0:VA], xs[r, :, 0:VA]).then_inc(sem_za[r], 16)
            else:
                eng.dma_start(zt[r % NZ][:, VA:V], xs[r, :, VA:V]).then_inc(sem_zb[r], 16)

        @block.sync
        def _(sync):
            sync.dma_start(wt[:], wbd[:]).then_inc(sem_w, 16)
            for r in range(0, R, 2):  # even rows: a then b
                issue_row_half(sync, r, 0, gated=True)
                issue_row_half(sync, r, 1, gated=False)
            finals = [(sem_w, 16), (sem_bias, 16), (sem_mm, NG), (sem_act, NG)]
            finals += [(s, 16 * (NG // OB // NOUT)) for s in sem_ob]
            finals += [(s, 16) for s in sem_za] + [(s, 16) for s in sem_zb]
            for s, v in finals:
                sync.wait_ge(s, v)

        @block.tensor
        def _(tensor):
            for g in range(NG):
                u, rnd = divmod(g, 2)
                if g == 0:
                    tensor.wait_ge(sem_w, 16)
                    tensor.wait_ge(sem_za[0], 16)
                    tensor.wait_ge(sem_za[1], 16)
                if g == 1:
                    tensor.wait_ge(sem_zb[0], 16)
                    tensor.wait_ge(sem_zb[1], 16)
                if u >= 1:
                    tensor.wait_ge((sem_za if rnd == 0 else sem_zb)[u + 1], 16)
                if g >= NPS:
                    tensor.wait_ge(sem_act, g - NPS + 1)
                psg = ps[g % NPS]
                valid = _shifts_for(u)
                last = None
                for s in valid:
                    ku, kv = SHIFTS[s]
                    for c in range(4):
                        v0 = (rnd * 4 + c) * VBS
                        vv0 = max(0, 1 - kv - v0)
                        vv1 = min(VBS, V + 1 - kv - v0)
                        a = v0 + vv0 + kv - 1
                        last = nc.tensor.matmul(
                            psg[c * 32 : (c + 1) * 32, vv0:vv1, :, :],
                            wt[:, s, :],
                            zt[(u + ku - 1) % NZ][:, a : a + (vv1 - vv0), :, :],
                            start=(s == valid[0]),
                            stop=(s == valid[-1]),
                            skip_group_check=True,
                            tile_position=(0, c * 32),
                        )
                last.then_inc(sem_mm)

        @block.scalar
        def _(scalar):
            scalar.dma_start(bt[:], bias[:]).then_inc(sem_bias, 16)
            odd = [r for r in range(R) if r % 2 == 1]
            for g in range(NG):
                # odd-row DMAs, interleaved by their WAR trigger (<= act trigger g+1)
                while odd and _row_gate(odd[0]) <= g + 1:
                    r = odd.pop(0)
                    issue_row_half(scalar, r, 0, gated=True)
                    issue_row_half(scalar, r, 1, gated=False)
                scalar.wait_ge(sem_mm, g + 1)
                if g == 0:
                    scalar.wait_ge(sem_bias, 16)
                bo = g // OB
                if bo >= NOUT and g % OB == 0:
                    scalar.wait_ge(sem_ob[bo % NOUT], 16 * (bo // NOUT))
                nc.scalar.activation(
                    ot[bo % NOUT][:, g % OB],
                    ps[g % NPS][:],
                    mybir.ActivationFunctionType.Identity,
                    bias=bt[:],
                ).then_inc(sem_act)
                if g % OB == OB - 1:
                    scalar.wait_ge(sem_act, OB * bo + OB)
                    scalar.dma_start(
                        ys[:, OB * bo : OB * bo + OB], ot[bo % NOUT][:]
                    ).then_inc(sem_ob[bo % NOUT], 16)

        blk_ctx.__exit__(None, None, None)

    nc.compile()
    return nc


def _unshard(results):
    y = np.empty((B, COUT, D1, D2, I, J), np.float32)
    for core in range(NCORES):
        bb, half = divmod(core, 2)
        arr = results[core]["ys"].astype(np.float32)
        # [128, g=(u,rnd), vb, io, j] ; p = 32c + 8cout + ib
        arr = arr.reshape(4, COUT, IB, U, 2, VBS, IO, J)
        arr = arr.transpose(1, 3, 4, 0, 5, 6, 2, 7)  # cout,u,rnd,c,vb,io,ib,j
        arr = arr.reshape(COUT, U, V, I, J)
        if half == 1:
            arr = arr[:, ::-1]
        y[bb, :, half * U : (half + 1) * U] = arr
    return y


TRACE = False
LAST_RESULT = [None]


def kernel(x, w, b, _cache={}):
    if "nc" not in _cache:
        _cache["nc"] = _build_program()
    nc = _cache["nc"]
    wbd_t, bias = _host_weights(w, b)
    wbd_f, bias_f = _host_weights(np.asarray(w)[:, :, ::-1], b)
    shards = _host_shard(x)
    in_maps = []
    for core in range(NCORES):
        half = core % 2
        in_maps.append({
            "xs": shards[core],
            "wbd": wbd_f if half == 1 else wbd_t,
            "bias": bias_f if half == 1 else bias,
        })
    res = run_bass_kernel_spmd(nc, in_maps, list(range(NCORES)), trace=TRACE)
    LAST_RESULT[0] = res
    return _unshard(res.results)
